# revision 26
# baseline (speedup 1.0000x reference)
"""DotAttention Trainium2 Bass kernel.

out[b] = softmax(Q[b] @ K[b]^T, axis=-1) @ K[b]
  Q: [16, 1024, 4096] f32, K: [16, 2048, 4096] f32 -> out [16, 1024, 4096] f32

Sharding: batch dim across 8 NeuronCores (2 batches/core), fully local.

Per-core pipeline (per batch), all matmuls fp16 with fp32 PSUM accumulation:
  0. Pre-pass: Q/K cast fp32->fp16 by SWDGE DRAM->DRAM DMAs on the (otherwise
     idle) GPSIMD queues into DRAM scratch; batch N+1's pre-pass runs under
     batch N's compute, so only batch 0 pays a cold start.
  1. Stage 1: Q^T and K^T quarter buffers built by xbar DMA-transposes
     reading the fp16 scratch.  At every batch start the critical pair
     (K^T quarter 0 + first Q quarter) goes first with a single xbar mode
     switch; the rest of Q follows as one large transfer under the first
     matmuls.
  2. Logits A = Q K^T per k-quarter (512 keys).  Online softmax: per-quarter
     (negated) local max m_q and sum s_q; e = exp(a - m_q) stored fp16 in
     E[q, k].
  3. Merge pass per q-tile: global max, corrections f_q = exp(m_q - m)
     rescale E (per-partition = per-query), r = 1/sum.
  4. C = E^T.T @ K: E rows xbar-transposed per q-tile; K-natural fp16 chunks
     DMA'd from scratch into the slots vacated by Q^T / K^T quarters.
     Normalization by r folds into the PSUM->SBUF copyback (ACT scale).

SBUF budget (per partition): 64KB Q^T/K-chunks slot + 2x32KB K^T-quarter
slots + 32KB E + ~40KB staging = ~200KB of the ~208KB usable.
"""

import numpy as np

import concourse.bass as bass
import concourse.bacc as bacc
import concourse.mybir as mybir
import concourse.tile as tile
from concourse.bass_utils import run_bass_kernel_spmd

P = 128
N_CORES = 8
B_FULL, LQ, LK, D = 16, 1024, 2048, 4096
B_PER_CORE = B_FULL // N_CORES  # 2

F16 = mybir.dt.float16
F32 = mybir.dt.float32
AX = mybir.AxisListType
AF = mybir.ActivationFunctionType


def build_program(b_per_core=B_PER_CORE, lq=LQ, lk=LK, d=D):
    nqt = lq // P          # q-tiles
    nkc = lk // P          # k-chunks
    nqtr = 4               # k-quarters for online softmax
    kc_per_qtr = nkc // nqtr
    qtr_k = lk // nqtr     # keys per quarter
    dc_n = d // P          # d-chunks
    dh_n = 2               # halves for loads/casts and second-matmul psum
    dhs = d // dh_n

    nc = bacc.Bacc("TRN2", target_bir_lowering=False, debug=False, num_swdge_queues=4)
    q_dram = nc.dram_tensor("query", [b_per_core, lq, d], F32, kind="ExternalInput").ap()
    k_dram = nc.dram_tensor("key", [b_per_core, lk, d], F32, kind="ExternalInput").ap()
    o_dram = nc.dram_tensor("out", [b_per_core, lq, d], F32, kind="ExternalOutput").ap()
    qf16 = nc.dram_tensor("qf16_scratch", [b_per_core, lq, d], F16, kind="Internal").ap()
    kf16 = nc.dram_tensor("kf16_scratch", [b_per_core, lk, d], F16, kind="Internal").ap()

    with tile.TileContext(nc) as tc:
        with (
            # 64KB/partition slot: Q^T during logits, then K-natural chunks 8..15
            tc.tile_pool(name="u64", bufs=1) as u64,
            # 2x 32KB/partition slots: K^T quarters (rotating), then K-natural 0..7
            tc.tile_pool(name="kt32", bufs=2) as kt32,
            # 32KB/partition: unscaled E [q, k] fp16
            tc.tile_pool(name="epool", bufs=1) as epool,
            # 8KB/partition: fp32 staging halves, fp16 row blocks, f32 out staging
            tc.tile_pool(name="s8", bufs=3) as s8,
            # 4KB/partition: E^T tiles for stage C
            tc.tile_pool(name="ettp", bufs=3) as ettp,
            tc.tile_pool(name="stats", bufs=2) as stats,
            tc.tile_pool(name="psum", bufs=2, space="PSUM") as psum,
        ):
            def prepass(b, src, dst, r0, r1):
                """fp32 -> fp16 cast during a SWDGE DRAM->DRAM DMA (GPSIMD
                queues -- parallel to the SP HWDGE ring)."""
                nc.gpsimd.dma_start(out=dst[b, r0:r1, :], in_=src[b, r0:r1, :])

            for b in range(b_per_core):
                # ---- stage 1: Q^T resident [P, dc, q] ----
                qt_full = u64.tile([P, dc_n, lq], F16, tag="u64", name=f"qtf_{b}")
                # critical path at every batch start: K^T quarter 0 and the
                # first Q quarter transpose first (one xbar mode switch), the
                # rest of Q as one big transfer under the first matmuls
                ktq0 = kt32.tile([P, dc_n, qtr_k], F16, tag="k32",
                                 name=f"ktq_{b}_0")
                if b == 0:
                    prepass(b, k_dram, kf16, 0, qtr_k)
                    prepass(b, q_dram, qf16, 0, lq // 4)
                nc.sync.dma_start_transpose(ktq0[:], kf16[b, 0:qtr_k, :])
                nc.sync.dma_start_transpose(
                    qt_full[:, :, 0:lq // 4], qf16[b, 0:lq // 4, :]
                )
                if b == 0:
                    prepass(b, q_dram, qf16, lq // 4, lq)
                nc.sync.dma_start_transpose(
                    qt_full[:, :, lq // 4:], qf16[b, lq // 4:, :]
                )

                # per-batch softmax stats
                M = stats.tile([P, nqt, nqtr], F32, tag="m", name=f"M_{b}")
                S = stats.tile([P, nqt, nqtr], F32, tag="s", name=f"S_{b}")
                F = stats.tile([P, nqt, nqtr], F32, tag="f", name=f"F_{b}")
                R = stats.tile([P, nqt], F32, tag="r", name=f"R_{b}")
                E = epool.tile([P, nqt, lk], F16, tag="e", name=f"E_{b}")

                # ---- logits + per-quarter softmax ----
                for q4 in range(nqtr):
                    if q4 == 0:
                        ktq = ktq0
                    else:
                        if b == 0:
                            prepass(b, k_dram, kf16, q4 * qtr_k,
                                    (q4 + 1) * qtr_k)
                        ktq = kt32.tile([P, dc_n, qtr_k], F16, tag="k32",
                                        name=f"ktq_{b}_{q4}")
                        nc.sync.dma_start_transpose(
                            ktq[:], kf16[b, q4 * qtr_k:(q4 + 1) * qtr_k, :]
                        )
                    for qt in range(nqt):
                        aps = psum.tile([P, qtr_k], F32, tag="ps",
                                        name=f"aps_{b}_{q4}_{qt}")
                        for dc in range(dc_n):
                            nc.tensor.matmul(
                                aps,
                                qt_full[:, dc, qt * P:(qt + 1) * P],
                                ktq[:, dc, :],
                                start=(dc == 0),
                                stop=(dc == dc_n - 1),
                            )
                        nc.vector.reduce_max(
                            M[:, qt, q4:q4 + 1], aps, axis=AX.X, negate=True
                        )
                        nc.scalar.activation(
                            E[:, qt, q4 * qtr_k:(q4 + 1) * qtr_k], aps, AF.Exp,
                            bias=M[:, qt, q4:q4 + 1], scale=1.0,
                            accum_out=S[:, qt, q4:q4 + 1],
                        )

                # ---- next batch's pre-pass overlaps this batch's compute ----
                if b + 1 < b_per_core:
                    prepass(b + 1, q_dram, qf16, 0, lq)
                    for q4 in range(nqtr):
                        prepass(b + 1, k_dram, kf16, q4 * qtr_k, (q4 + 1) * qtr_k)

                # ---- merge pass ----
                for qt in range(nqt):
                    negm = stats.tile([P, 1], F32, tag="negm", name=f"negm_{b}_{qt}")
                    nc.vector.tensor_reduce(
                        negm, M[:, qt, :], axis=AX.X, op=mybir.AluOpType.min
                    )
                    nc.scalar.activation(
                        F[:, qt, :], M[:, qt, :], AF.Exp, bias=negm, scale=-1.0
                    )
                    fs = stats.tile([P, nqtr], F32, tag="fs", name=f"fs_{b}_{qt}")
                    nc.vector.tensor_mul(fs, F[:, qt, :], S[:, qt, :])
                    sg = stats.tile([P, 1], F32, tag="sg", name=f"sg_{b}_{qt}")
                    nc.vector.reduce_sum(sg, fs, axis=AX.X)
                    nc.vector.reciprocal(R[:, qt:qt + 1], sg)
                    for q4 in range(nqtr):
                        sl = E[:, qt, q4 * qtr_k:(q4 + 1) * qtr_k]
                        nc.vector.tensor_scalar_mul(sl, sl, F[:, qt, q4:q4 + 1])

                # ---- second matmul: C = E^T.T @ K ----
                knB0 = kt32.tile([P, kc_per_qtr, d], F16, tag="k32", name=f"knB0_{b}")
                knB1 = kt32.tile([P, kc_per_qtr, d], F16, tag="k32", name=f"knB1_{b}")
                knA = u64.tile([P, nkc - 2 * kc_per_qtr, d], F16, tag="u64",
                               name=f"knA_{b}")

                def kn_chunk(kc):
                    if kc < kc_per_qtr:
                        return knB0[:, kc, :]
                    if kc < 2 * kc_per_qtr:
                        return knB1[:, kc - kc_per_qtr, :]
                    return knA[:, kc - 2 * kc_per_qtr, :]

                for kc in range(nkc):
                    # SWDGE queue: keeps the SP ring free for xposes/stores
                    nc.gpsimd.dma_start(
                        out=kn_chunk(kc), in_=kf16[b, kc * P:(kc + 1) * P, :]
                    )

                for qt in range(nqt):
                    et_t = ettp.tile([P, nkc, P], F16, tag="ett", name=f"ett_{b}_{qt}")
                    nc.sync.dma_start_transpose(et_t, E[:, qt, :])
                    if qt == nqt - 1:
                        # final q-tile: read knB0 (kt32 slot 0) and knA (u64)
                        # early, knB1 last -- frees the slots the next batch's
                        # critical K^T/Q^T transposes need ~10us before this
                        # batch's last matmul retires.  PSUM accumulation
                        # order is associative-free here.
                        korder = (list(range(kc_per_qtr))
                                  + list(range(2 * kc_per_qtr, nkc))
                                  + list(range(kc_per_qtr, 2 * kc_per_qtr)))
                    else:
                        korder = list(range(nkc))
                    for dh in range(dh_n):
                        cps = psum.tile([P, dhs], F32, tag="ps",
                                        name=f"cps_{b}_{qt}_{dh}")
                        for i, kc in enumerate(korder):
                            for nb in range(dhs // 512):
                                nc.tensor.matmul(
                                    cps[:, nb * 512:(nb + 1) * 512],
                                    et_t[:, kc, :],
                                    kn_chunk(kc)[:, dh * dhs + nb * 512:
                                                 dh * dhs + (nb + 1) * 512],
                                    start=(i == 0),
                                    stop=(i == nkc - 1),
                                )
                        c_out = s8.tile([P, dhs], F32, tag="s8", name=f"co_{b}_{qt}_{dh}")
                        nc.scalar.mul(c_out, cps, R[:, qt:qt + 1])
                        nc.sync.dma_start(
                            out=o_dram[b, qt * P:(qt + 1) * P, dh * dhs:(dh + 1) * dhs],
                            in_=c_out,
                        )
    nc.compile()
    return nc


_PROGRAM = None


def _get_program():
    global _PROGRAM
    if _PROGRAM is None:
        _PROGRAM = build_program()
    return _PROGRAM


LAST_RESULTS = None  # BassKernelResults of the most recent kernel() call


def kernel(query: np.ndarray, key: np.ndarray) -> np.ndarray:
    global LAST_RESULTS
    query = np.ascontiguousarray(query, dtype=np.float32)
    key = np.ascontiguousarray(key, dtype=np.float32)
    assert query.shape == (B_FULL, LQ, D), query.shape
    assert key.shape == (B_FULL, LK, D), key.shape

    nc = _get_program()
    in_maps = [
        {
            "query": np.ascontiguousarray(query[i * B_PER_CORE:(i + 1) * B_PER_CORE]),
            "key": np.ascontiguousarray(key[i * B_PER_CORE:(i + 1) * B_PER_CORE]),
        }
        for i in range(N_CORES)
    ]
    res = run_bass_kernel_spmd(nc, in_maps, core_ids=list(range(N_CORES)))
    LAST_RESULTS = res
    out = np.concatenate([r["out"] for r in res.results], axis=0)
    return np.ascontiguousarray(out.astype(np.float32))
